# revision 24
# baseline (speedup 1.0000x reference)
"""Trainium2 Bass kernel for the MiniGRU cell (B=131072 rows, data-parallel over 8 cores).

Math (per row b):
    tokens = concat(stoch, action) @ proj_w + proj_b            # [256]
    parts  = LN(concat(tokens, deter) @ core_w) * g + b         # [768]
    reset, cand_in, upd_in = split(parts, 3)
    reset = sigmoid(reset); cand = tanh(reset * cand_in); upd = sigmoid(upd_in - 1)
    out = upd * cand + (1 - upd) * deter                        # [256]

Host-side folding: both matmuls collapse into one x_aug @ W_c where
x_aug = [stoch, deter, action, 1] (401 features, zero-padded to 512) and
W_c has its per-row column-mean removed so the LayerNorm mean subtraction
is built into the matmul (mean(q) == 0 up to rounding); the device only
computes rstd = 1/sqrt(mean(q^2) + eps) (RMS-style).  W_c is additionally
Frobenius-normalized so mean(q^2) ~= 1, which lets a 2-step Newton rsqrt
(from y0=1) replace the ACT Sqrt table (whose set excludes Sigmoid/Tanh).

Device layout: activations are fed feature-major (host pre-transposed) as the
matmul's stationary operand, weights stream as the moving operand, so the
matmul output lands batch-major in PSUM ([128 rows, 768]) where LN stats are a
free-axis reduction and rstd is a per-partition scalar (usable as ACT scale).

v2 schedule (pair-section pipeline, one section = 2 batch tiles = 256 rows):
  section s:  MMs(pair s) + bn_stats/Square/aggr(pair s) + newton(pair s)
              gates(pair s-1): sigmoid x2 per tile (ACT, scale=rstd, reads
              PSUM directly = fused evacuation) + stt per tile (DVE,
              (q_c*rstd)*sig_r -> tt1, also evacuates the cand third)
              every 2nd section: tanh(quad) [ACT] + blend [GPSIMD:
              d1=cand-det, d2=upd*d1, outb=d2+det] + store of an older quad.
  Newton rsqrt chain is pair-batched (FD=2) to amortize DVE fixed costs.
  The whole blend lives on GPSIMD (it is otherwise idle; DVE is the
  critical engine).  Lagging gates one section keeps every engine's strict
  FIFO stall-free: when ACT reaches pair s-1's sigmoids, their rstd was
  produced during pair s's matmul time.

Measured (loop-slope, R=8193): 439787 ns baseline -> 248784 ns for this
schedule.  Variants tried and rejected:
  - MM_ORDER=kouter (k-outer matmuls, 2 open PSUM accumulation groups):
    488766 ns — walrus does not dedupe repeated LDWEIGHTS and interleaved
    groups stall the PE.
  - FP8=1 (e4m3 DoubleRow, K=256 super-chunks): numerically correct
    (rel 1.358e-2, host-sim-predicted) but 501650-520746 ns with either
    W layout — the DoubleRow lowering runs ~2x slower than bf16 here.
"""

import os
import sys

for _p in ("/opt/trn_rl_repo",):
    if _p not in sys.path and os.path.isdir(_p):
        sys.path.insert(0, _p)

import numpy as np
import ml_dtypes

from contextlib import ExitStack

import concourse.bass as bass
import concourse.bacc as bacc
import concourse.tile as tile
from concourse import mybir
from concourse.bass_utils import run_bass_kernel_spmd

BF16 = ml_dtypes.bfloat16

B_FULL = 131072
DETER = 256
STOCH = 128
ACT_DIM = 16
HID = 256
NOUT = 3 * DETER          # 768
N_CORES = 8
BC = B_FULL // N_CORES    # 16384 rows per core
KPAD = 512                # padded contraction dim: [stoch 128 | deter 256 | action 16 | ones 1 | zeros 111]
LN_EPS = 1e-5

OCT = 1024                # batch rows per DMA slab (8 tiles of 128 = 4 pair-sections)
N_OCT = BC // OCT         # 16

_F32 = mybir.dt.float32
_BF16 = mybir.dt.bfloat16

# Matmul N-chunks (offset, n): 768 outputs as 512+256, each within one PSUM bank.
_CHUNKS = ((0, 512), (512, 256))

# LN stats split: DVE bn_stats covers q[:, :BN_N], ACT Square-accum covers the rest.
BN_N = int(os.environ.get("BN_N", "512"))
SQ_N = NOUT - BN_N

# MM emission order: "region" = chunk-outer (bn_stats input finishes early),
# "kouter" = k-outer (each stationary x-tile feeds 2 consecutive matmuls).
MM_ORDER = os.environ.get("MM_ORDER", "region")

# fp8 DoubleRow matmul: x and W quantized to e4m3 (scaled by SX/SW), PE runs
# 2 fp8 MACs/cell/cycle with the 128x128 array virtualized to K=256.  The
# Newton-rsqrt chain is re-seeded so rstd comes out in PSUM units (scale
# invariance of LN makes the quantization scales drop out exactly).
FP8 = os.environ.get("FP8", "0") == "1"
SX = 16.0                 # activation quant scale
SW = 64.0                 # weight quant scale
C_INV = 1.0 / (SX * SW)   # rstd unit conversion, folded into the Newton seed
E4M3 = ml_dtypes.float8_e4m3
_FP8 = mybir.dt.float8e4

_last_results = None  # BassKernelResults of the most recent run (for profiling)


def build_nc(bc: int = BC, loop: int = 1) -> bass.Bass:
    """Build the per-core Bass program. All 8 cores run this same program.

    `loop` > 1 wraps the whole per-core body in a hardware For_i that re-runs
    it `loop` times over the same data — used only by the timing bench.
    """
    n_oct = bc // OCT
    nc = bacc.Bacc("TRN2", target_bir_lowering=False, debug=False, num_devices=1)

    if FP8:
        # x: [ki, sc, ko, b] (ko-planes contiguous — LDW loads plane 0 then 1)
        # W: [ki, sc, n, ko] (ko-pair adjacent per output column — the moving
        #    streamer fetches both K-rows of a column in one 2-byte read)
        # effective contraction row = 256*sc + 128*ko + ki
        xw = nc.declare_dram_parameter("xw", [128, 2, 2, bc], _FP8, isOutput=False)
        wts = nc.declare_dram_parameter("wts", [128, 2, NOUT, 2], _FP8, isOutput=False)
    else:
        xw = nc.declare_dram_parameter("xw", [128, 4, bc], _BF16, isOutput=False)
        wts = nc.declare_dram_parameter("wts", [128, 4, NOUT], _BF16, isOutput=False)
    det = nc.declare_dram_parameter("det", [bc, DETER], _BF16, isOutput=False)
    out = nc.declare_dram_parameter("out", [bc, DETER], _BF16, isOutput=True)

    with tile.TileContext(nc) as tc, ExitStack() as ctx:
        singles = ctx.enter_context(tc.tile_pool(name="singles", bufs=1))
        xpool = ctx.enter_context(tc.tile_pool(name="x", bufs=3))
        dpool = ctx.enter_context(tc.tile_pool(name="det", bufs=3))
        spool = ctx.enter_context(tc.tile_pool(name="stats", bufs=4))
        gpool = ctx.enter_context(tc.tile_pool(name="gates", bufs=3))
        sqpool = ctx.enter_context(tc.tile_pool(name="sq", bufs=4))
        qpool = ctx.enter_context(tc.tile_pool(name="q", bufs=4, space="PSUM"))

        if FP8:
            w_t = singles.tile([128, 2, NOUT, 2], _FP8)
        else:
            w_t = singles.tile([128, 4, NOUT], _BF16)
        nc.sync.dma_start(w_t[:], wts[:])
        neg1_t = singles.tile([128, 1], _F32)
        nc.vector.memset(neg1_t[:], -1.0)

        pools = dict(xpool=xpool, dpool=dpool, spool=spool, gpool=gpool,
                     sqpool=sqpool, qpool=qpool, neg1=neg1_t)

        def body():
            st = _PipeState()
            prefetch(nc, pools, xw, det, 0, st)
            for o in range(n_oct):
                if o + 1 < n_oct:
                    prefetch(nc, pools, xw, det, o + 1, st)
                for p in range(4):
                    section(nc, pools, w_t, out, o, p, st)
            flush(nc, pools, out, st)

        if loop > 1:
            with tc.For_i(0, loop):
                body()
        else:
            body()

    nc.finalize()
    return nc


class _PipeState:
    """Deferred stages for software pipelining across section emission."""

    def __init__(self):
        self.prev_pair = None     # pair record awaiting gates (lag 1 section)
        self.pending_quads = []   # quad records awaiting tanh+blend+store
        self.cur_quad = None      # quad record being filled by gates stages
        self.xdet = {}            # oct -> (x_t, det_t) prefetched input tiles


def prefetch(nc, pools, xw, det, o, st):
    """DMA oct o's activations + deter slab into SBUF (one oct ahead).

    Input DMAs ride the ACT HWDGE queue; store DMAs (which wait on late
    compute) ride the SP queue, so inputs are never stuck behind stores."""
    if FP8:
        x_t = pools["xpool"].tile([128, 2, 2, OCT], _FP8, name=f"x_{o}", tag="x")
        nc.scalar.dma_start(x_t[:], xw[:, :, :, o * OCT:(o + 1) * OCT])
    else:
        x_t = pools["xpool"].tile([128, 4, OCT], _BF16, name=f"x_{o}", tag="x")
        nc.scalar.dma_start(x_t[:], xw[:, :, o * OCT:(o + 1) * OCT])
    det_t = pools["dpool"].tile([128, 8, DETER], _BF16, name=f"dt_{o}", tag="det")
    nc.scalar.dma_start(
        det_t[:], det[o * OCT:(o + 1) * OCT, :].rearrange("(t p) f -> p t f", p=128)
    )
    st.xdet[o] = (x_t, det_t)


def _emit_bn_stats(nc, st6, j, qt):
    assert BN_N <= 512
    nc.vector.bn_stats(st6[:, j, 0:6], qt[:, 0:BN_N])


def section(nc, pools, w_t, out, o, p, st):
    """One pair-section: 2 batch tiles of 128 rows."""
    spool, gpool, sqpool, qpool = (
        pools["spool"], pools["gpool"], pools["sqpool"], pools["qpool"])
    x_t, det_t = st.xdet[o]
    if p == 3:
        st.xdet.pop(o)

    # ---- stage C/D: tanh + blend + store for the oldest completed quad.
    # Emitted FIRST so ACT consumes the tanh (inputs long ready) before
    # stalling on anything, and GPSIMD/SP run free.
    if st.pending_quads and st.pending_quads[0]["ready"]:
        emit_quad_tail(nc, pools, out, st.pending_quads.pop(0))

    # ---- stage B: gates for the previous pair (rstd computed last section).
    if st.prev_pair is not None:
        emit_gates(nc, pools, st.prev_pair, st)

    # ---- stage A: matmuls + LN stats for this pair.
    st6 = spool.tile([128, 2, 9], _F32, name=f"st6_{o}_{p}", tag="st6")
    mv = spool.tile([128, 2, 2], _F32, name=f"mv_{o}_{p}", tag="mv")
    y1 = spool.tile([128, 2], _F32, name=f"y1_{o}_{p}", tag="y1")
    t2 = spool.tile([128, 2], _F32, name=f"t2_{o}_{p}", tag="t2")
    rstd = spool.tile([128, 2], _F32, name=f"rstd_{o}_{p}", tag="rstd")
    if o == 0:
        # slots 6 (count) and 7 (mean) of the synthetic Square group are
        # constants; spool rotates with period 4 so o==0's four sections
        # initialize every physical buffer once for the whole pass
        nc.vector.memset(st6[:, :, 6], float(SQ_N))
        nc.vector.memset(st6[:, :, 7], 0.0)

    qts = []
    for j in range(2):
        t = 2 * p + j
        qt = qpool.tile([128, NOUT], _F32, name=f"q{o}_{t}", tag="q")  # 2 banks
        qts.append(qt)
        lhs_cols = slice(t * 128, (t + 1) * 128)
        if FP8:
            # 2 DoubleRow super-chunks of K=256 each; chunk-outer so each
            # PSUM bank's accumulation group closes before the next opens,
            # and the [0:512] bank finishes early for bn_stats.
            for (qo, n) in _CHUNKS:
                for sc in range(2):
                    nc.tensor.matmul(
                        qt[:, qo:qo + n],
                        x_t[:, sc, :, lhs_cols],
                        w_t[:, sc, qo:qo + n, :].rearrange("p n k -> p k n"),
                        start=(sc == 0),
                        stop=(sc == 1),
                        perf_mode=mybir.MatmulPerfMode.DoubleRow,
                    )
                if qo == 0:
                    _emit_bn_stats(nc, st6, j, qt)
        elif MM_ORDER == "kouter":
            for k in range(4):
                for (qo, n) in _CHUNKS:
                    nc.tensor.matmul(
                        qt[:, qo:qo + n],
                        x_t[:, k, lhs_cols],
                        w_t[:, k, qo:qo + n],
                        start=(k == 0),
                        stop=(k == 3),
                    )
            _emit_bn_stats(nc, st6, j, qt)
        else:
            # region-major: the [0:512] bank finishes 4 matmuls earlier,
            # letting bn_stats (the longest stats op) start sooner
            for (qo, n) in _CHUNKS:
                for k in range(4):
                    nc.tensor.matmul(
                        qt[:, qo:qo + n],
                        x_t[:, k, lhs_cols],
                        w_t[:, k, qo:qo + n],
                        start=(k == 0),
                        stop=(k == 3),
                    )
                if qo == 0:
                    _emit_bn_stats(nc, st6, j, qt)
        sqs = sqpool.tile([128, SQ_N], _BF16, name=f"sqs_{o}_{t}", tag="sqs")
        nc.scalar.activation(
            out=sqs[:], in_=qt[:, BN_N:NOUT],
            func=mybir.ActivationFunctionType.Square,
            accum_out=st6[:, j, 8:9],
        )
        nc.vector.bn_aggr(mv[:, j, :], st6[:, j, :])

    # rstd for both tiles in one pair-batched Newton chain (FD=2).  With fp8,
    # v is in PSUM units ((SX*SW)^2 too large); substituting y1 -> c*y1' with
    # c = 1/(SX*SW) folds the unit conversion into the seed coefficients and
    # the chain emerges as rstd in PSUM units — exactly what the PSUM-reading
    # gates need (LayerNorm is scale invariant):
    #   y1 = c*(1.5 - 0.5*c^2*v);  rstd = y1*(1.5 - 0.5*v*y1^2)
    c = C_INV if FP8 else 1.0
    var = mv[:, :, 1]
    nc.vector.tensor_scalar(
        out=y1[:], in0=var, scalar1=-0.5 * c * c * c, scalar2=1.5 * c,
        op0=mybir.AluOpType.mult, op1=mybir.AluOpType.add,
    )
    nc.vector.tensor_mul(t2[:], y1[:], y1[:])       # y1^2
    nc.vector.tensor_mul(t2[:], var, t2[:])         # v*y1^2
    nc.vector.tensor_scalar(
        out=t2[:], in0=t2[:], scalar1=-0.5, scalar2=1.5,
        op0=mybir.AluOpType.mult, op1=mybir.AluOpType.add,
    )
    nc.vector.tensor_mul(rstd[:], t2[:], y1[:])

    g = 4 * o + p
    st.prev_pair = dict(o=o, p=p, g=g, qts=qts, rstd=rstd, det_t=det_t)


def emit_gates(nc, pools, pr, st):
    """Stage B for pair `pr`: two sigmoids per tile (ACT, PSUM-direct with
    scale=rstd) + stt per tile (DVE) writing into quad-level upd/tt1 tiles."""
    gpool = pools["gpool"]
    g = pr["g"]
    q = g // 2
    if g % 2 == 0:
        # first pair of the quad: allocate the quad-level gate tiles
        upd = gpool.tile([128, 4, DETER], _BF16, name=f"upd_{q}", tag="upd")
        tt1 = gpool.tile([128, 4, DETER], _BF16, name=f"tt1_{q}", tag="tt1")
        st.cur_quad = dict(q=q, o=pr["o"], upd=upd, tt1=tt1,
                           det_t=pr["det_t"], ready=False)
    quad = st.cur_quad
    upd, tt1 = quad["upd"], quad["tt1"]

    sigr = gpool.tile([128, 2, DETER], _BF16, name=f"sig_{g}", tag="sig")
    for j in range(2):
        qt = pr["qts"][j]
        r_ = pr["rstd"][:, j:j + 1]
        tq = 2 * (g % 2) + j
        nc.scalar.activation(
            out=sigr[:, j], in_=qt[:, 0:DETER],
            func=mybir.ActivationFunctionType.Sigmoid, scale=r_,
        )
        nc.scalar.activation(
            out=upd[:, tq], in_=qt[:, 2 * DETER:3 * DETER],
            func=mybir.ActivationFunctionType.Sigmoid, scale=r_,
            bias=pools["neg1"][:],
        )
        nc.vector.scalar_tensor_tensor(
            out=tt1[:, tq], in0=qt[:, DETER:2 * DETER],
            scalar=r_, in1=sigr[:, j],
            op0=mybir.AluOpType.mult, op1=mybir.AluOpType.mult,
        )                                            # (q_c*rstd)*sig_r

    if g % 2 == 1:
        quad["ready"] = True
        st.pending_quads.append(quad)
        st.cur_quad = None
    st.prev_pair = None


def emit_quad_tail(nc, pools, out, quad):
    """Stage C: tanh (ACT) + blend (GPSIMD) + store (SP) for one quad."""
    gpool = pools["gpool"]
    q = quad["q"]
    qq = q % 2                               # quad within its oct
    dslc = quad["det_t"][:, 4 * qq:4 * qq + 4]
    cand = gpool.tile([128, 4, DETER], _BF16, name=f"cand_{q}", tag="cand")
    d1 = gpool.tile([128, 4, DETER], _BF16, name=f"d1_{q}", tag="d1")
    d2 = gpool.tile([128, 4, DETER], _BF16, name=f"d2_{q}", tag="d2")
    outb = gpool.tile([128, 4, DETER], _BF16, name=f"outb_{q}", tag="outb")
    nc.scalar.activation(
        out=cand[:], in_=quad["tt1"][:], func=mybir.ActivationFunctionType.Tanh,
    )
    nc.gpsimd.tensor_sub(d1[:], cand[:], dslc)           # cand - det
    nc.gpsimd.tensor_mul(d2[:], quad["upd"][:], d1[:])   # upd * (cand - det)
    nc.gpsimd.tensor_add(outb[:], d2[:], dslc)           # + det
    base = q * 512
    nc.sync.dma_start(
        out[base:base + 512, :].rearrange("(t p) f -> p t f", p=128),
        outb[:],
    )


def flush(nc, pools, out, st):
    """Drain the deferred pipeline stages at the end of the program."""
    if st.prev_pair is not None:
        emit_gates(nc, pools, st.prev_pair, st)
    while st.pending_quads:
        emit_quad_tail(nc, pools, out, st.pending_quads.pop(0))


_nc_cache: dict[tuple, bass.Bass] = {}


def _get_nc(bc: int) -> bass.Bass:
    key = (bc, MM_ORDER, FP8)
    if key not in _nc_cache:
        _nc_cache[key] = build_nc(bc)
    return _nc_cache[key]


def _fold_weights(proj_w, proj_b, core_w):
    """Collapse both matmuls + LN mean-subtraction into one [KPAD, 768] matrix,
    Frobenius-normalized so mean over the 768 outputs of q^2 is ~1."""
    W1 = proj_w.astype(np.float64) @ core_w[:HID].astype(np.float64)   # [144, 768]
    W2 = core_w[HID:].astype(np.float64)                               # [256, 768]
    b1 = proj_b.astype(np.float64) @ core_w[:HID].astype(np.float64)   # [768]
    W_all = np.zeros((KPAD, NOUT), np.float64)
    W_all[0:STOCH] = W1[:STOCH]
    W_all[STOCH:STOCH + DETER] = W2
    W_all[STOCH + DETER:STOCH + DETER + ACT_DIM] = W1[STOCH:]
    W_all[STOCH + DETER + ACT_DIM] = b1
    # remove per-row column mean -> mean_j(x @ W_c) == 0 exactly
    W_c = W_all - W_all.mean(axis=1, keepdims=True)
    # normalize so E[mean_j q_j^2] == 1 (Newton rsqrt converges from y0=1;
    # LayerNorm output is invariant to this scale)
    W_c *= np.sqrt(NOUT / np.square(W_c).sum())
    return W_c


def prepare_in_maps(deter, stoch, action, proj_w, proj_b, core_w):
    """Host-side folding/packing shared by kernel() and bench.py."""
    B = deter.shape[0]
    assert B % N_CORES == 0
    bc = B // N_CORES

    W_c = _fold_weights(proj_w, proj_b, core_w)

    # Feature-major activations, padded to KPAD rows: [stoch; deter; action; ones; zeros]
    xf = np.empty((KPAD, B), np.float32)
    xf[0:STOCH] = stoch.T
    xf[STOCH:STOCH + DETER] = deter.T
    xf[STOCH + DETER:STOCH + DETER + ACT_DIM] = action.T
    xf[STOCH + DETER + ACT_DIM] = 1.0
    xf[STOCH + DETER + ACT_DIM + 1:] = 0.0

    if FP8:
        # contraction row = 256*sc + 128*ko + ki
        wp = np.ascontiguousarray(
            (W_c * SW).reshape(2, 2, 128, NOUT).transpose(2, 0, 3, 1)
        ).astype(E4M3)                                                # [128, 2, 768, 2]
        xb = np.ascontiguousarray(
            (xf * SX).reshape(2, 2, 128, B).transpose(2, 0, 1, 3)
        ).astype(E4M3)                                                # [128, 2, 2, B]
    else:
        wp = np.ascontiguousarray(
            W_c.reshape(4, 128, NOUT).transpose(1, 0, 2)).astype(BF16)  # [128, 4, 768]
        xb = np.ascontiguousarray(
            xf.astype(BF16).reshape(4, 128, B).transpose(1, 0, 2))      # [128, 4, B]

    det_b = deter.astype(BF16)

    in_maps = []
    for c in range(N_CORES):
        in_maps.append({
            "xw": np.ascontiguousarray(xb[..., c * bc:(c + 1) * bc]),
            "wts": wp,
            "det": np.ascontiguousarray(det_b[c * bc:(c + 1) * bc]),
        })
    return in_maps


def kernel(deter, stoch, action, proj_w, proj_b, core_w, ln_g, ln_b):
    global _last_results
    deter = np.asarray(deter, np.float32)
    stoch = np.asarray(stoch, np.float32)
    action = np.asarray(action, np.float32)
    proj_w = np.asarray(proj_w, np.float32)
    proj_b = np.asarray(proj_b, np.float32)
    core_w = np.asarray(core_w, np.float32)
    ln_g = np.asarray(ln_g, np.float32)
    ln_b = np.asarray(ln_b, np.float32)

    if not (np.allclose(ln_g, 1.0) and np.allclose(ln_b, 0.0)):
        # General-affine LN is not wired into the device fast path; fall back to
        # exact host math (setup_inputs always passes g=1, b=0 so this is unused).
        return _host_reference(deter, stoch, action, proj_w, proj_b, core_w, ln_g, ln_b)

    B = deter.shape[0]
    bc = B // N_CORES
    in_maps = prepare_in_maps(deter, stoch, action, proj_w, proj_b, core_w)

    nc = _get_nc(bc)
    res = run_bass_kernel_spmd(nc, in_maps, core_ids=list(range(N_CORES)))
    _last_results = res
    return np.concatenate(
        [res.results[c]["out"] for c in range(N_CORES)], axis=0
    ).astype(np.float32)


def _host_reference(deter, stoch, action, proj_w, proj_b, core_w, ln_g, ln_b):
    x = np.concatenate([stoch, action], axis=-1) @ proj_w + proj_b
    parts = np.concatenate([x, deter], axis=-1) @ core_w
    mu = parts.mean(-1, keepdims=True)
    var = ((parts - mu) ** 2).mean(-1, keepdims=True)
    parts = (parts - mu) / np.sqrt(var + LN_EPS) * ln_g + ln_b
    d = parts.shape[-1] // 3
    reset = 1.0 / (1.0 + np.exp(-parts[..., :d]))
    cand = np.tanh(reset * parts[..., d:2 * d])
    upd = 1.0 / (1.0 + np.exp(-(parts[..., 2 * d:] - 1.0)))
    return (upd * cand + (1.0 - upd) * deter).astype(np.float32)
